# revision 5
# baseline (speedup 1.0000x reference)
"""Trainium2 Bass kernel for nn_GCL2D (contrastive PDE loss).

Strategy (8 NeuronCores, H-band sharding):
  Each core c owns H rows [16c, 16c+16). It loads only its band's rows of
  x1/x2 (with +-1 halo rows and +-1 W-halo columns pre-wrapped on host) and
  u (no halo), keeping every DMA fully contiguous per (b,h) row.

  On-chip per core:
    - extract t=0 slice of x-rows / t=8,9 slices of u-rows (strided ACT copy)
    - upwind-advection + diffusion stencil:
        y = z + A0*lap(z) - z*M(z)
      H-direction shift terms are folded into two per-tile [128,128]
      matrices (built on host from vs/dt/dx) applied on the tensor engine;
      W-direction terms use shifted free-dim APs in fused
      scalar_tensor_tensor ops with per-partition coefficient columns.
    - PE transposes to [w-partition, (h,a)-free] layout
    - accumulating PE matmuls produce the partial Gram matrix
      [Y|Z]^T [Y|Z] (64x64) and [U8|U9]^T [U8|U9] (32x32) over the band.
  Host sums the 8 partial Grams (8 x 24KB) and evaluates the final O(B^2)
  scalar formula (pairwise Frobenius norms via Gram identity, contrastive
  combine) - the device does all O(B*H*W) work.
"""

import numpy as np

import concourse.bass as bass
import concourse.tile as tile
from concourse import bacc, mybir
from concourse.bass_utils import run_bass_kernel_spmd

F32 = mybir.dt.float32
ALU = mybir.AluOpType

B, H, W, T = 16, 128, 128, 10
NCORES = 8
BAND = H // NCORES            # 16 rows per core
HLO = BAND + 2                # 18 rows incl halo
WP = W + 2                    # 130 cols incl W-halo
RXROWS = B * HLO              # 288
RXF = 2 * WP * T              # 2600 (x2-half | x1-half)
RUROWS = B * BAND             # 256
RUF = W * T                   # 1280
TAU = 100.0

# x-tiles: 7,7,2 b-blocks of 18 rows
XNB = (7, 7, 2)
XROWS = tuple(nb * HLO for nb in XNB)      # 126,126,36
XBASE = (0, 126, 252)
XBB = (0, 7, 14)

_PROG = None


def _build_program():
    nc = bacc.Bacc("TRN2", target_bir_lowering=False, debug=False)

    rx_d = nc.dram_tensor("rx", [RXROWS, RXF], F32, kind="ExternalInput")
    ru_d = nc.dram_tensor("ru", [RUROWS, RUF], F32, kind="ExternalInput")
    pm_d = nc.dram_tensor("pm", [128, 6 * 128], F32, kind="ExternalInput")
    cf_d = nc.dram_tensor("cf", [128, 9], F32, kind="ExternalInput")
    idn_d = nc.dram_tensor("idn", [128, 128], F32, kind="ExternalInput")
    po_d = nc.dram_tensor("po", [64, 96], F32, kind="ExternalOutput")

    with tile.TileContext(nc) as tc:
        with (
            tc.tile_pool(name="const", bufs=1) as cpool,
            tc.tile_pool(name="raw", bufs=2) as rawp,
            tc.tile_pool(name="work", bufs=2) as wp,
            tc.tile_pool(name="acc", bufs=1) as accp,
            tc.tile_pool(name="ps_st", bufs=1, space=bass.MemorySpace.PSUM) as ps_st,
            tc.tile_pool(name="ps_tp", bufs=1, space=bass.MemorySpace.PSUM) as ps_tp,
            tc.tile_pool(name="ps_gr", bufs=1, space=bass.MemorySpace.PSUM) as ps_gr,
        ):
            pm = cpool.tile([128, 6 * 128], F32)
            cf = cpool.tile([128, 9], F32)
            idn = cpool.tile([128, 128], F32)
            nc.sync.dma_start(pm[:], pm_d[:])
            nc.sync.dma_start(cf[:], cf_d[:])
            nc.sync.dma_start(idn[:], idn_d[:])

            # Accumulation targets in the [w-part, (h, a)] layout
            yz = accp.tile([128, 16 * 64], F32)   # a: 32 y-cols | 32 z-cols
            uu = accp.tile([128, 16 * 32], F32)   # a: 16 u8 | 16 u9

            # ---- x stencil tiles ----
            for t in range(3):
                rows, nb = XROWS[t], XNB[t]
                raw = rawp.tile([128, RXF], F32, tag="rawx")
                nc.sync.dma_start(raw[0:rows, :], rx_d[XBASE[t]:XBASE[t] + rows, :])

                zp = wp.tile([128, 2 * WP], F32, tag="zp")
                nc.gpsimd.memset(zp[:], 0.0)
                src = raw[0:rows].rearrange("p (q w t) -> p q w t", q=2, w=WP, t=T)[:, :, :, 0]
                nc.scalar.copy(zp[0:rows].rearrange("p (q k) -> p q k", q=2), src)

                pe1 = ps_st.tile([128, 2 * WP], F32, tag="pe1")
                pe2 = ps_st.tile([128, 2 * WP], F32, tag="pe2")
                nc.tensor.matmul(pe1[:], pm[:, (2 * t) * 128:(2 * t + 1) * 128], zp[:], start=True, stop=True)
                nc.tensor.matmul(pe2[:], pm[:, (2 * t + 1) * 128:(2 * t + 2) * 128], zp[:], start=True, stop=True)

                zq = zp[0:rows].rearrange("p (q k) -> p q k", q=2)
                zm1, zin, zp1 = zq[:, :, 0:W], zq[:, :, 1:W + 1], zq[:, :, 2:W + 2]
                pe1v = pe1[0:rows].rearrange("p (q k) -> p q k", q=2)[:, :, 1:W + 1]
                pe2v = pe2[0:rows].rearrange("p (q k) -> p q k", q=2)[:, :, 1:W + 1]

                def qv(tl):
                    return tl[0:rows].rearrange("p (q k) -> p q k", q=2)

                sws = wp.tile([128, 2 * W], F32, tag="sws")
                nc.vector.tensor_tensor(qv(sws), zm1, zp1, ALU.add)
                s1 = wp.tile([128, 2 * W], F32, tag="s1")
                nc.vector.scalar_tensor_tensor(qv(s1), qv(sws), cf[0:rows, 3 * t:3 * t + 1], pe1v, ALU.mult, ALU.add)
                mp = wp.tile([128, 2 * W], F32, tag="mp")
                nc.vector.scalar_tensor_tensor(qv(mp), zp1, cf[0:rows, 3 * t + 1:3 * t + 2], pe2v, ALU.mult, ALU.add)
                mm = wp.tile([128, 2 * W], F32, tag="mm")
                nc.vector.scalar_tensor_tensor(qv(mm), zm1, cf[0:rows, 3 * t + 2:3 * t + 3], qv(mp), ALU.mult, ALU.add)
                zmul = wp.tile([128, 2 * W], F32, tag="zmul")
                nc.gpsimd.tensor_tensor(qv(zmul), zin, qv(mm), ALU.mult)
                yt = wp.tile([128, 2 * W], F32, tag="yt")
                nc.gpsimd.tensor_tensor(qv(yt), qv(s1), qv(zmul), ALU.subtract)

                # transposes + scatter into yz
                for q in range(2):
                    abase = q * 16 + XBB[t]
                    ty = ps_tp.tile([128, 128], F32, tag="ty")
                    nc.tensor.transpose(ty[0:128, 0:rows], yt[0:rows, q * W:(q + 1) * W], idn[0:rows, 0:rows])
                    nc.vector.tensor_copy(
                        yz[:].rearrange("p (h a) -> p a h", h=16, a=64)[:, abase:abase + nb, :],
                        ty[:, 0:rows].rearrange("p (j h) -> p j h", j=nb, h=HLO)[:, :, 1:17],
                    )
                    tz = ps_tp.tile([128, 128], F32, tag="tz")
                    nc.tensor.transpose(tz[0:128, 0:rows], zp[0:rows, q * WP + 1:q * WP + 1 + W], idn[0:rows, 0:rows])
                    nc.scalar.copy(
                        yz[:].rearrange("p (h a) -> p a h", h=16, a=64)[:, 32 + abase:32 + abase + nb, :],
                        tz[:, 0:rows].rearrange("p (j h) -> p j h", j=nb, h=HLO)[:, :, 1:17],
                    )

            # ---- u tiles ----
            for tu in range(2):
                rawu = rawp.tile([128, RUF], F32, tag="rawu")
                nc.sync.dma_start(rawu[:], ru_d[tu * 128:(tu + 1) * 128, :])
                for s, coff in ((8, 0), (9, 16)):
                    tp_ps = ps_tp.tile([128, 128], F32, tag="tu")
                    nc.tensor.transpose(
                        tp_ps[:],
                        rawu[:].rearrange("p (w t) -> p w t", t=T)[:, :, s],
                        idn[:],
                    )
                    nc.vector.tensor_copy(
                        uu[:].rearrange("p (h a) -> p a h", h=16, a=32)[:, coff + tu * 8:coff + tu * 8 + 8, :],
                        tp_ps[:].rearrange("p (j h) -> p j h", j=8, h=16),
                    )

            # ---- grams ----
            g64 = ps_gr.tile([64, 64], F32, tag="g64")
            yzv = yz[:].rearrange("p (h a) -> p h a", h=16)
            for h in range(16):
                nc.tensor.matmul(g64[:], yzv[:, h, :], yzv[:, h, :], start=(h == 0), stop=(h == 15))
            g32 = ps_gr.tile([32, 32], F32, tag="g32")
            uuv = uu[:].rearrange("p (h a) -> p h a", h=16)
            for h in range(16):
                nc.tensor.matmul(g32[:], uuv[:, h, :], uuv[:, h, :], start=(h == 0), stop=(h == 15))

            po = cpool.tile([64, 96], F32)
            nc.gpsimd.memset(po[:], 0.0)
            nc.vector.tensor_copy(po[:, 0:64], g64[:])
            nc.vector.tensor_copy(po[0:32, 64:96], g32[:])
            nc.sync.dma_start(po_d[:], po[:])

    nc.compile()
    return nc


def _get_prog():
    global _PROG
    if _PROG is None:
        _PROG = _build_program()
    return _PROG


def _host_inputs(x1, x2, vs, u, dt, dxf):
    dtv = dt.reshape(B)
    c0, c1, c2 = vs[:, 0], vs[:, 1], vs[:, 2]
    A0 = c0 / np.float32(dxf) ** 2
    r = dtv / np.float32(dxf)
    am = np.where(c1 <= 0, -c1, 0).astype(np.float32) * r
    ap = np.where(c1 > 0, c1, 0).astype(np.float32) * r
    bp = np.where(c2 >= 0, c2, 0).astype(np.float32) * r
    bm = np.where(c2 < 0, -c2, 0).astype(np.float32) * r
    casum = am + ap + bm + bp

    pm = np.zeros((128, 6 * 128), np.float32)
    cf = np.zeros((128, 9), np.float32)
    for t in range(3):
        nb = XNB[t]
        M1 = np.zeros((128, 128), np.float32)
        M2 = np.zeros((128, 128), np.float32)
        for j in range(nb):
            b = XBB[t] + j
            base = j * HLO
            for hl in range(HLO):
                rr = base + hl
                M1[rr, rr] = 1.0 - 4.0 * A0[b]
                M2[rr, rr] = casum[b]
                if hl + 1 < HLO:
                    M1[rr, rr + 1] = A0[b]      # SH+ term of lap
                    M2[rr, rr + 1] = -bm[b]     # -bm * SH+
                if hl - 1 >= 0:
                    M1[rr, rr - 1] = A0[b]      # SH-
                    M2[rr, rr - 1] = -bp[b]     # -bp * SH-
                cf[rr, 3 * t] = A0[b]
                cf[rr, 3 * t + 1] = -am[b]
                cf[rr, 3 * t + 2] = -ap[b]
        pm[:, (2 * t) * 128:(2 * t + 1) * 128] = M1.T
        pm[:, (2 * t + 1) * 128:(2 * t + 2) * 128] = M2.T

    idn = np.eye(128, dtype=np.float32)

    in_maps = []
    for c in range(NCORES):
        s = BAND * c
        hrows = np.arange(s - 1, s + BAND + 1) % H
        rxs = []
        for xb in (x2, x1):
            xbnd = xb[:, hrows]                                   # [16,18,128,10]
            xpad = np.concatenate([xbnd[:, :, W - 1:W], xbnd, xbnd[:, :, 0:1]], axis=2)
            rxs.append(np.ascontiguousarray(xpad.reshape(RXROWS, WP * T)))
        rx = np.concatenate(rxs, axis=1)                          # [288, 2600]
        ru = np.ascontiguousarray(u[:, s:s + BAND].reshape(RUROWS, RUF))
        in_maps.append({"rx": rx, "ru": ru, "pm": pm, "cf": cf, "idn": idn})
    return in_maps


def _finish(po_sum, vs):
    G64 = po_sum[:, 0:64].astype(np.float64)
    G32 = po_sum[0:32, 64:96].astype(np.float64)
    ny = np.diag(G64)[0:32]
    nz = np.diag(G64)[32:64]
    Gyz = G64[0:32, 32:64]
    pd = np.sqrt(np.maximum(ny[:, None] + nz[None, :] - 2.0 * Gyz, 0.0))
    n8 = np.diag(G32)[0:16]
    n9 = np.diag(G32)[16:32]
    Gu = G32[0:16, 16:32]
    us0 = np.sqrt(np.maximum(n8[:, None] + n9[None, :] - 2.0 * Gu, 0.0))
    us = np.tile(us0, (2, 2))
    sim = (us - pd) ** 2

    t = np.stack([vs[:, 0], np.linalg.norm(vs[:, 1:], axis=1)], axis=-1).astype(np.float64)
    prod = np.sqrt(np.sum(np.abs(t[None, :, :] * t[:, None, :]), axis=-1))
    nv = np.linalg.norm(t, axis=-1)
    nm = np.maximum(nv[:, None], nv[None, :])
    S = np.tile(prod / nm, (2, 2))

    pos = 0.5 * S * sim ** 2
    scal = max(float((TAU - sim ** 2).max()), 0.0)
    neg = 0.5 * (1.0 - S) * scal
    return np.float32((pos + neg).mean() / B)


def kernel(x1, x2, vs, u, dt, dx):
    x1 = np.asarray(x1, np.float32)
    x2 = np.asarray(x2, np.float32)
    vs = np.asarray(vs, np.float32)
    u = np.asarray(u, np.float32)
    dt = np.asarray(dt, np.float32)
    dxf = float(np.asarray(dx))

    prog = _get_prog()
    in_maps = _host_inputs(x1, x2, vs, u, dt, dxf)
    res = run_bass_kernel_spmd(prog, in_maps, list(range(NCORES))).results
    po_sum = np.zeros((64, 96), np.float64)
    for r in res:
        po_sum += r["po"].astype(np.float64)
    return np.asarray(_finish(po_sum, vs), dtype=np.float32)


# revision 9
# speedup vs baseline: 1.0087x; 1.0087x over previous
"""Trainium2 Bass kernel for nn_GCL2D (contrastive PDE loss).

Strategy (8 NeuronCores, H-band sharding):
  Each core c owns H rows [16c, 16c+16). It loads only its band's rows of
  x1/x2 (with +-1 halo rows and +-1 W-halo columns pre-wrapped on host) and
  u (no halo), keeping every DMA fully contiguous per (b,h) row.

  On-chip per core:
    - extract t=0 slice of x-rows / t=8,9 slices of u-rows (strided ACT copy)
    - upwind-advection + diffusion stencil:
        y = z + A0*lap(z) - z*M(z)
      H-direction shift terms are folded into two per-tile [128,128]
      matrices (built on host from vs/dt/dx) applied on the tensor engine
      as float32r matmuls (full precision, full rate at >=256 moving dim);
      W-direction terms use shifted free-dim APs in fused
      scalar_tensor_tensor ops with per-partition coefficient columns.
    - bf16 PE transposes to one [w-partition, (h, a)-free] tile holding
      Y|Z|U8|U9 slabs, then 16 accumulating bf16 PE matmuls produce the
      partial Gram matrix [Y|Z|U8|U9]^T [...] (96x96, fp32 PSUM) over the
      band.
  Host sums the 8 partial Grams (8 x 36KB) and evaluates the final O(B^2)
  scalar formula (pairwise Frobenius norms via the Gram identity,
  contrastive combine) - the device does all O(B*H*W) work.
"""

import numpy as np
import ml_dtypes

import concourse.bass as bass
import concourse.tile as tile
from concourse import bacc, mybir
from concourse.bass_utils import run_bass_kernel_spmd

F32 = mybir.dt.float32
F32R = mybir.dt.float32r
BF16 = mybir.dt.bfloat16
ALU = mybir.AluOpType

B, H, W, T = 16, 128, 128, 10
NCORES = 8
BAND = H // NCORES            # 16 rows per core
HLO = BAND + 2                # 18 rows incl halo
WP = W + 2                    # 130 cols incl W-halo
RXROWS = B * HLO              # 288
RXF = 2 * WP * T              # 2600 (x2-half | x1-half)
RUROWS = B * BAND             # 256
RUF = W * T                   # 1280
TAU = 100.0

# x-tiles: 7,7,2 b-blocks of 18 rows
XNB = (7, 7, 2)
XROWS = tuple(nb * HLO for nb in XNB)      # 126,126,36
XBASE = (0, 126, 252)
XBB = (0, 7, 14)

_PROG = None


def _build_program():
    nc = bacc.Bacc("TRN2", target_bir_lowering=False, debug=False)

    rx_d = nc.dram_tensor("rx", [RXROWS, RXF], F32, kind="ExternalInput")
    ru_d = nc.dram_tensor("ru", [RUROWS, RUF], F32, kind="ExternalInput")
    pm_d = nc.dram_tensor("pm", [128, 6 * 128], F32, kind="ExternalInput")
    cf_d = nc.dram_tensor("cf", [128, 9], F32, kind="ExternalInput")
    idn_d = nc.dram_tensor("idn", [128, 128], BF16, kind="ExternalInput")
    po_d = nc.dram_tensor("po", [96, 96], F32, kind="ExternalOutput")

    with tile.TileContext(nc) as tc:
        with (
            tc.tile_pool(name="const", bufs=1) as cpool,
            tc.tile_pool(name="raw", bufs=2) as rawp,
            tc.tile_pool(name="work", bufs=2) as wp,
            tc.tile_pool(name="acc", bufs=1) as accp,
            tc.tile_pool(name="ps_st", bufs=1, space=bass.MemorySpace.PSUM) as ps_st,
            tc.tile_pool(name="ps_tp", bufs=2, space=bass.MemorySpace.PSUM) as ps_tp,
            tc.tile_pool(name="ps_gr", bufs=1, space=bass.MemorySpace.PSUM) as ps_gr,
        ):
            pm = cpool.tile([128, 6 * 128], F32)
            cf = cpool.tile([128, 9], F32)
            idn = cpool.tile([128, 128], BF16)
            nc.sync.dma_start(pm[:], pm_d[:])
            nc.sync.dma_start(cf[:], cf_d[:])
            nc.sync.dma_start(idn[:], idn_d[:])
            pmr = cpool.tile([128, 6 * 128], F32R)
            nc.scalar.copy(pmr[:], pm[:])

            # accumulation target: [w-part, (h, a)] with a = Y(32)|Z(32)|U8(16)|U9(16)
            a2 = accp.tile([128, 16 * 96], BF16)

            # ---- x stencil tiles ----
            for t in range(3):
                rows, nb = XROWS[t], XNB[t]
                raw = rawp.tile([128, RXF], F32, tag="rawx")
                nc.sync.dma_start(raw[0:rows, :], rx_d[XBASE[t]:XBASE[t] + rows, :])

                zp = wp.tile([128, 2 * WP], F32, tag="zp")
                nc.gpsimd.memset(zp[:], 0.0)
                src = raw[0:rows].rearrange("p (q w t) -> p q w t", q=2, w=WP, t=T)[:, :, :, 0]
                nc.scalar.copy(zp[0:rows].rearrange("p (q k) -> p q k", q=2), src)

                zr = wp.tile([128, 2 * WP], F32R, tag="zr")
                nc.scalar.copy(zr[:], zp[:])
                pe1 = ps_st.tile([128, 2 * WP], F32, tag="pe1")
                pe2 = ps_st.tile([128, 2 * WP], F32, tag="pe2")
                nc.tensor.matmul(pe1[:], pmr[:, (2 * t) * 128:(2 * t + 1) * 128],
                                 zr[:], start=True, stop=True)
                nc.tensor.matmul(pe2[:], pmr[:, (2 * t + 1) * 128:(2 * t + 2) * 128],
                                 zr[:], start=True, stop=True)

                zq = zp[0:rows].rearrange("p (q k) -> p q k", q=2)
                zm1, zin, zp1 = zq[:, :, 0:W], zq[:, :, 1:W + 1], zq[:, :, 2:W + 2]
                pe1v = pe1[0:rows].rearrange("p (q k) -> p q k", q=2)[:, :, 1:W + 1]
                pe2v = pe2[0:rows].rearrange("p (q k) -> p q k", q=2)[:, :, 1:W + 1]

                def qv(tl):
                    return tl[0:rows].rearrange("p (q k) -> p q k", q=2)

                sws = wp.tile([128, 2 * W], F32, tag="sws")
                nc.vector.tensor_tensor(qv(sws), zm1, zp1, ALU.add)
                s1 = wp.tile([128, 2 * W], F32, tag="s1")
                nc.vector.scalar_tensor_tensor(qv(s1), qv(sws), cf[0:rows, 3 * t:3 * t + 1], pe1v, ALU.mult, ALU.add)
                mp = wp.tile([128, 2 * W], F32, tag="mp")
                nc.vector.scalar_tensor_tensor(qv(mp), zp1, cf[0:rows, 3 * t + 1:3 * t + 2], pe2v, ALU.mult, ALU.add)
                mm = wp.tile([128, 2 * W], F32, tag="mm")
                nc.vector.scalar_tensor_tensor(qv(mm), zm1, cf[0:rows, 3 * t + 2:3 * t + 3], qv(mp), ALU.mult, ALU.add)
                zmul = wp.tile([128, 2 * W], F32, tag="zmul")
                nc.gpsimd.tensor_tensor(qv(zmul), zin, qv(mm), ALU.mult)
                yt = wp.tile([128, 2 * W], F32, tag="yt")
                nc.gpsimd.tensor_tensor(qv(yt), qv(s1), qv(zmul), ALU.subtract)

                # bf16 casts for the gram path
                yb = wp.tile([128, 2 * W], BF16, tag="yb")
                nc.scalar.copy(yb[0:rows, :], yt[0:rows, :])
                zb = wp.tile([128, 2 * W], BF16, tag="zb")
                nc.scalar.copy(zb[0:rows].rearrange("p (q k) -> p q k", q=2), zin)

                # transposes + scatter into a2
                for q in range(2):
                    abase = q * 16 + XBB[t]
                    ty = ps_tp.tile([128, 128], BF16, tag="ty")
                    nc.tensor.transpose(ty[0:128, 0:rows], yb[0:rows, q * W:(q + 1) * W], idn[0:rows, 0:rows])
                    nc.vector.tensor_copy(
                        a2[:].rearrange("p (h a) -> p a h", h=16, a=96)[:, abase:abase + nb, :],
                        ty[:, 0:rows].rearrange("p (j h) -> p j h", j=nb, h=HLO)[:, :, 1:17],
                    )
                    tz = ps_tp.tile([128, 128], BF16, tag="tz")
                    nc.tensor.transpose(tz[0:128, 0:rows], zb[0:rows, q * W:(q + 1) * W], idn[0:rows, 0:rows])
                    nc.scalar.copy(
                        a2[:].rearrange("p (h a) -> p a h", h=16, a=96)[:, 32 + abase:32 + abase + nb, :],
                        tz[:, 0:rows].rearrange("p (j h) -> p j h", j=nb, h=HLO)[:, :, 1:17],
                    )

            # ---- u tiles ----
            for tu in range(2):
                rawu = rawp.tile([128, RUF], F32, tag="rawu")
                nc.sync.dma_start(rawu[:], ru_d[tu * 128:(tu + 1) * 128, :])
                for s, coff in ((8, 64), (9, 80)):
                    ub = wp.tile([128, 128], BF16, tag="ub")
                    nc.scalar.copy(ub[:], rawu[:].rearrange("p (w t) -> p w t", t=T)[:, :, s])
                    tp_ps = ps_tp.tile([128, 128], BF16, tag="ty")
                    nc.tensor.transpose(tp_ps[:], ub[:], idn[:])
                    nc.vector.tensor_copy(
                        a2[:].rearrange("p (h a) -> p a h", h=16, a=96)[:, coff + tu * 8:coff + tu * 8 + 8, :],
                        tp_ps[:].rearrange("p (j h) -> p j h", j=8, h=16),
                    )

            # ---- merged gram ----
            g96 = ps_gr.tile([96, 96], F32, tag="g96")
            a2v = a2[:].rearrange("p (h a) -> p h a", h=16)
            for h in range(16):
                nc.tensor.matmul(g96[:], a2v[:, h, :], a2v[:, h, :], start=(h == 0), stop=(h == 15))

            po = cpool.tile([96, 96], F32)
            nc.vector.tensor_copy(po[:], g96[:])
            nc.sync.dma_start(po_d[:], po[:])

    nc.compile()
    return nc


def _get_prog():
    global _PROG
    if _PROG is None:
        _PROG = _build_program()
    return _PROG


def _host_inputs(x1, x2, vs, u, dt, dxf):
    dtv = dt.reshape(B)
    c0, c1, c2 = vs[:, 0], vs[:, 1], vs[:, 2]
    A0 = c0 / np.float32(dxf) ** 2
    r = dtv / np.float32(dxf)
    am = np.where(c1 <= 0, -c1, 0).astype(np.float32) * r
    ap = np.where(c1 > 0, c1, 0).astype(np.float32) * r
    bp = np.where(c2 >= 0, c2, 0).astype(np.float32) * r
    bm = np.where(c2 < 0, -c2, 0).astype(np.float32) * r
    casum = am + ap + bm + bp

    pm = np.zeros((128, 6 * 128), np.float32)
    cf = np.zeros((128, 9), np.float32)
    for t in range(3):
        nb = XNB[t]
        M1 = np.zeros((128, 128), np.float32)
        M2 = np.zeros((128, 128), np.float32)
        for j in range(nb):
            b = XBB[t] + j
            base = j * HLO
            for hl in range(HLO):
                rr = base + hl
                M1[rr, rr] = 1.0 - 4.0 * A0[b]
                M2[rr, rr] = casum[b]
                if hl + 1 < HLO:
                    M1[rr, rr + 1] = A0[b]      # SH+ term of lap
                    M2[rr, rr + 1] = -bm[b]     # -bm * SH+
                if hl - 1 >= 0:
                    M1[rr, rr - 1] = A0[b]      # SH-
                    M2[rr, rr - 1] = -bp[b]     # -bp * SH-
                cf[rr, 3 * t] = A0[b]
                cf[rr, 3 * t + 1] = -am[b]
                cf[rr, 3 * t + 2] = -ap[b]
        pm[:, (2 * t) * 128:(2 * t + 1) * 128] = M1.T
        pm[:, (2 * t + 1) * 128:(2 * t + 2) * 128] = M2.T

    idn = np.eye(128).astype(ml_dtypes.bfloat16)

    in_maps = []
    for c in range(NCORES):
        s = BAND * c
        hrows = np.arange(s - 1, s + BAND + 1) % H
        rxs = []
        for xb in (x2, x1):
            xbnd = xb[:, hrows]                                   # [16,18,128,10]
            xpad = np.concatenate([xbnd[:, :, W - 1:W], xbnd, xbnd[:, :, 0:1]], axis=2)
            rxs.append(np.ascontiguousarray(xpad.reshape(RXROWS, WP * T)))
        rx = np.concatenate(rxs, axis=1)                          # [288, 2600]
        ru = np.ascontiguousarray(u[:, s:s + BAND].reshape(RUROWS, RUF))
        in_maps.append({"rx": rx, "ru": ru, "pm": pm, "cf": cf, "idn": idn})
    return in_maps


def _finish(po_sum, vs):
    G = po_sum.astype(np.float64)
    ny = np.diag(G)[0:32]
    nz = np.diag(G)[32:64]
    Gyz = G[0:32, 32:64]
    pd = np.sqrt(np.maximum(ny[:, None] + nz[None, :] - 2.0 * Gyz, 0.0))
    n8 = np.diag(G)[64:80]
    n9 = np.diag(G)[80:96]
    Gu = G[64:80, 80:96]
    us0 = np.sqrt(np.maximum(n8[:, None] + n9[None, :] - 2.0 * Gu, 0.0))
    us = np.tile(us0, (2, 2))
    sim = (us - pd) ** 2

    t = np.stack([vs[:, 0], np.linalg.norm(vs[:, 1:], axis=1)], axis=-1).astype(np.float64)
    prod = np.sqrt(np.sum(np.abs(t[None, :, :] * t[:, None, :]), axis=-1))
    nv = np.linalg.norm(t, axis=-1)
    nm = np.maximum(nv[:, None], nv[None, :])
    S = np.tile(prod / nm, (2, 2))

    pos = 0.5 * S * sim ** 2
    scal = max(float((TAU - sim ** 2).max()), 0.0)
    neg = 0.5 * (1.0 - S) * scal
    return np.float32((pos + neg).mean() / B)


def kernel(x1, x2, vs, u, dt, dx):
    x1 = np.asarray(x1, np.float32)
    x2 = np.asarray(x2, np.float32)
    vs = np.asarray(vs, np.float32)
    u = np.asarray(u, np.float32)
    dt = np.asarray(dt, np.float32)
    dxf = float(np.asarray(dx))

    prog = _get_prog()
    in_maps = _host_inputs(x1, x2, vs, u, dt, dxf)
    res = run_bass_kernel_spmd(prog, in_maps, list(range(NCORES))).results
    po_sum = np.zeros((96, 96), np.float64)
    for r in res:
        po_sum += r["po"].astype(np.float64)
    return np.asarray(_finish(po_sum, vs), dtype=np.float32)
